# revision 1
# baseline (speedup 1.0000x reference)
"""PET tube-of-response backprojection on 8 TRN2 NeuronCores, v2.

Slice-sharded as v1 (core c owns 16 z-slices of all three backprojections;
no collective). The per-slice scatter is computed as W_x^T @ W_y on the PE,
but unlike v1 the dense W tiles are built with large batched instructions:

  A_s[p, kf]  = (f - u_s[p,k])^2      one rank-64 PE matmul per side
                                      (exact-bf16 split coefficient rows)
  G_s         = Exp(-c*A_s [+ ln proj folded into coef])   one ACT op/side
  DI          = f - round(u)          one Pool tensor_tensor (exact ints)
  M01         = |DI| <= 1.5           one DVE tensor_scalar (4x mode)
  W           = G * M01               one DVE tensor_tensor (2x mode)
  PSUM_S[k]  += Wx^T @ Wy             8 PE matmuls

Per-(LOR,slice) coefficients (u, u^2 split into exact bf16 rows) are
precomputed once per axis into a coefficient slab, transposed per pass by
the PE into matmul-LHS layout.
"""

import math
import sys

sys.path.insert(0, "/opt/trn_rl_repo")
sys.path.insert(0, "/opt/trn_rl_repo/concourse")

import numpy as np

V = 1.5625
INV_V = float(np.float32(0.64))
SIGMA2 = 9.0 * math.pi / 4.0
C = 0.5 * V * V / SIGMA2          # exponent scale: w = exp(-C*(f-u)^2)
MAGIC = 12582912.0                # 1.5 * 2^23: round-to-nearest-even trick

N_CORES = 8
N_K = 16          # slices per core
KG = 8            # slices per pass (PSUM capacity)
N_CHUNKS = 128    # 128-LOR chunks
N_LORS = N_CHUNKS * 128

ROTATIONS = {"x": [1, 2, 0], "y": [0, 2, 1], "z": [0, 1, 2]}
BACK_ROTATIONS_IMAGE = {"x": [1, 2, 0], "y": [1, 0, 2], "z": [0, 1, 2]}
AXES = ("x", "y", "z")

_CACHE = {}


def _bf16(x):
    """Round fp32 array to bf16, keep as fp32."""
    u = np.asarray(x, np.float32).view(np.uint32)
    r = ((u >> 16) & 1).astype(np.uint32)
    out = ((u + 0x7FFF + r) >> 16) << 16
    return out.astype(np.uint32).view(np.float32)


def _host_consts():
    """Static tensors shared by all cores."""
    f = np.arange(128, dtype=np.float32)
    f2 = f * f
    f2hi = np.floor(f2 / 64.0) * 64.0          # exact bf16 (8-bit mantissa)
    f2lo = f2 - f2hi                           # 0..63, exact
    # RHS [64, 1024]: row r = k*8 + j, col = k'*128 + f_pos; nonzero iff k'==k
    pat = np.stack([f2hi, f2lo, f, f, f,
                    np.ones(128, np.float32),
                    np.ones(128, np.float32),
                    np.ones(128, np.float32)])  # [8, 128]
    rhs = np.zeros((128, 1024), np.float32)
    for k in range(8):
        rhs[k * 8:(k + 1) * 8, k * 128:(k + 1) * 128] = pat
    rhs[64:] = rhs[:64]
    # IOTA2: [128, 16, 128] -> flattened [128, 2048]: f pattern per block
    iota2 = np.broadcast_to(f, (128, 16, 128)).reshape(128, 2048).copy()
    ident = np.broadcast_to(np.eye(128, dtype=np.float32), (128, 128)).copy()
    return rhs, iota2, ident


def _host_tvals():
    zc = np.float32(-100.0) + (np.arange(128, dtype=np.float32)
                               + np.float32(0.5)) * np.float32(1.5625)
    return (zc + np.float32(100.0)) / np.float32(200.0)


def _build_kernel(repeat=1):
    from concourse import mybir, tile, bacc

    DT = mybir.dt
    F32 = DT.float32
    BF16 = DT.bfloat16
    AO = mybir.AluOpType
    AF = mybir.ActivationFunctionType
    n_axes = 3

    nc = bacc.Bacc("TRN2", target_bir_lowering=False, debug=False)
    lors_d = [nc.dram_tensor(f"lors{a}", [4, N_LORS], F32, kind="ExternalInput")
              for a in range(n_axes)]
    proj_d = [nc.dram_tensor(f"proj{a}", [N_LORS], F32, kind="ExternalInput")
              for a in range(n_axes)]
    rhs_d = nc.dram_tensor("rhs64", [128, 1024], BF16, kind="ExternalInput")
    iota2_d = nc.dram_tensor("iota2", [128, 2048], BF16, kind="ExternalInput")
    ident_d = nc.dram_tensor("ident", [128, 128], BF16, kind="ExternalInput")
    bigi_d = nc.dram_tensor("bigi", [128, 128], BF16, kind="ExternalInput")
    tval_d = nc.dram_tensor("tvals", [128, N_K], F32, kind="ExternalInput")
    slab_d = [nc.dram_tensor(f"slab{a}", [128, N_K, 128], F32,
                             kind="ExternalOutput") for a in range(n_axes)]

    with tile.TileContext(nc) as tc:
        with (
            tc.tile_pool(name="const", bufs=1) as constp,
            tc.tile_pool(name="coef", bufs=1) as coefp,
            tc.tile_pool(name="pre", bufs=1) as prep,
            tc.tile_pool(name="work", bufs=2) as workp,
            tc.tile_pool(name="lhs", bufs=2) as lhsp,
            tc.tile_pool(name="out", bufs=2) as outp,
            tc.tile_pool(name="psA", bufs=1, space="PSUM") as psA,
            tc.tile_pool(name="psT", bufs=2, space="PSUM") as psT,
            tc.tile_pool(name="psS", bufs=1, space="PSUM") as psS,
        ):
            # --- static tiles ---
            RHS = constp.tile([128, 1024], BF16, tag="rhs")
            nc.sync.dma_start(RHS[:], rhs_d[:])
            IOTA2 = constp.tile([128, 2048], BF16, tag="iota2")
            nc.sync.dma_start(IOTA2[:], iota2_d[:])
            IDENT = constp.tile([128, 128], BF16, tag="ident")
            nc.sync.dma_start(IDENT[:], ident_d[:])
            BIGI = constp.tile([128, 128], BF16, tag="bigi")
            nc.sync.dma_start(BIGI[:], bigi_d[:])
            TT = constp.tile([128, N_K], F32, tag="tt")
            nc.sync.dma_start(TT[:], tval_d[:])

            rep_ctx = tc.For_i(0, repeat, 1) if repeat > 1 else None
            if rep_ctx is not None:
                rep_ctx.__enter__()
            for a in range(n_axes):
                # ---------- prep phase ----------
                # coefficient slab CFP [128, c, kg, s*64 + k8*8 + j] bf16
                CFP = coefp.tile([128, N_CHUNKS, 2, 128], BF16, tag="cfp")
                # ix0 per (c, kg, s, k8) bf16 for the mask
                IX0B = coefp.tile([128, N_CHUNKS, 2, 2, 8], BF16, tag="ix0b")

                comp = []
                for r in range(4):
                    t_ = prep.tile([128, N_CHUNKS], F32, tag=f"comp{r}")
                    nc.sync.dma_start(
                        t_[:], lors_d[a][r, :].rearrange("(p c) -> p c", p=128))
                    comp.append(t_)
                P1X, P1Y, P2X, P2Y = comp
                PRJ = prep.tile([128, N_CHUNKS], F32, tag="prj")
                nc.sync.dma_start(PRJ[:],
                                  proj_d[a][:].rearrange("(p c) -> p c", p=128))
                LNPC = prep.tile([128, N_CHUNKS], F32, tag="lnpc")
                # -ln(proj)/C  (added to u^2 coefficient on the y side)
                nc.scalar.activation(LNPC[:], PRJ[:], AF.Ln)
                nc.vector.tensor_scalar(LNPC[:], LNPC[:], -1.0 / C, None,
                                        op0=AO.mult)

                for s, (P1, P2) in enumerate(((P1X, P2X), (P1Y, P2Y))):
                    DX = prep.tile([128, N_CHUNKS], F32, tag="dx")
                    nc.vector.tensor_tensor(DX[:], P2[:], P1[:], op=AO.subtract)
                    tb = TT[:].unsqueeze(1).broadcast_to([128, N_CHUNKS, N_K])
                    dxb = DX[:].unsqueeze(2).broadcast_to([128, N_CHUNKS, N_K])
                    p1b = P1[:].unsqueeze(2).broadcast_to([128, N_CHUNKS, N_K])
                    T1 = prep.tile([128, N_CHUNKS, N_K], F32, tag="t1")
                    T2 = prep.tile([128, N_CHUNKS, N_K], F32, tag="t2")
                    T3 = prep.tile([128, N_CHUNKS, N_K], F32, tag="t3")
                    T4 = prep.tile([128, N_CHUNKS, N_K], F32, tag="t4")
                    CX = T1
                    nc.vector.tensor_tensor(CX[:], tb, dxb, op=AO.mult)
                    nc.vector.tensor_tensor(CX[:], CX[:], p1b, op=AO.add)
                    # y = cx + 100 ; exact fp32 division by 1.5625 via
                    # q = y*0.64; r = ((y-q)-0.5q)-0.0625q; u' = q + r*0.64
                    Y_ = T2
                    nc.vector.tensor_scalar(Y_[:], CX[:], 100.0, None, op0=AO.add)
                    Q_ = T3
                    nc.vector.tensor_scalar(Q_[:], Y_[:], INV_V, None, op0=AO.mult)
                    R_ = T1
                    nc.vector.tensor_tensor(R_[:], Y_[:], Q_[:], op=AO.subtract)
                    nc.vector.scalar_tensor_tensor(R_[:], Q_[:], -0.5, R_[:],
                                                   op0=AO.mult, op1=AO.add)
                    nc.vector.scalar_tensor_tensor(R_[:], Q_[:], -0.0625, R_[:],
                                                   op0=AO.mult, op1=AO.add)
                    U = T2
                    nc.vector.scalar_tensor_tensor(U[:], R_[:], INV_V, Q_[:],
                                                   op0=AO.mult, op1=AO.add)
                    nc.vector.tensor_scalar(U[:], U[:], 0.5, None, op0=AO.subtract)
                    # ix0 = round-to-even(u)
                    IX0 = T3
                    nc.vector.tensor_scalar(IX0[:], U[:], MAGIC, MAGIC,
                                            op0=AO.add, op1=AO.subtract)
                    nc.vector.tensor_copy(
                        IX0B[:, :, :, s, :],
                        IX0[:].rearrange("p c (g k) -> p c g k", g=2))
                    # q2 = u^2 (+ -ln(proj)/C on the y side)
                    Q2 = T3
                    nc.vector.tensor_tensor(Q2[:], U[:], U[:], op=AO.mult)
                    if s == 1:
                        lb = LNPC[:].unsqueeze(2).broadcast_to(
                            [128, N_CHUNKS, N_K])
                        nc.vector.tensor_tensor(Q2[:], Q2[:], lb, op=AO.add)

                    # 3-way exact-residual bf16 splits of (-2u) and q2,
                    # written straight into the coefficient slab.
                    # CFP free layout: [c, kg, r] with r = s*64 + k8*8 + j
                    # CFP viewed [p, c, kg, s', k8, j]
                    cfpv = CFP[:].rearrange(
                        "p c g (t k j) -> p c g t k j", t=2, k=8)

                    SPB = prep.tile([128, N_CHUNKS, N_K], BF16, tag="spb")

                    def split3(SRC, scratch, scale0, jbase):
                        nc.vector.tensor_scalar(scratch[0][:], SRC[:], scale0,
                                                None, op0=AO.mult)
                        cur, spare = scratch
                        for lvl in range(3):
                            nc.vector.tensor_copy(
                                cfpv[:, :, :, s, :, jbase + lvl],
                                cur[:].rearrange("p c (g k) -> p c g k", g=2))
                            if lvl == 2:
                                break
                            nc.vector.tensor_copy(SPB[:], cur[:])
                            nc.vector.tensor_tensor(spare[:], cur[:], SPB[:],
                                                    op=AO.subtract)
                            cur, spare = spare, cur

                    split3(U, (T4, T1), -2.0, 2)   # j = 2, 3, 4
                    split3(Q2, (T4, T1), 1.0, 5)   # j = 5, 6, 7
                    # j = 0, 1: constant 1.0 rows (pair with f2hi, f2lo)
                    nc.vector.memset(cfpv[:, :, :, s, :, 0:2], 1.0)

                # ---------- main loop ----------
                # Software-pipelined: at pass n the engines run
                #   Pool: DI(n+1) | DVE: LHS-copy(n+1), CL(n+1), PB(n+1)
                #   PE: T(n+1), A(n)+penalty(n), S(n-1) | ACT: Exp(n)
                # so no engine waits on a same-pass producer.
                passes = [(kg, c) for kg in range(2) for c in range(N_CHUNKS)]

                def stage_mask(kg, c):
                    PT = psT.tile([128, 128], BF16, tag="pt")
                    nc.tensor.transpose(PT[:], CFP[:, c, kg, :], IDENT[:])
                    LHS = lhsp.tile([128, 128], BF16, tag="lhs")
                    nc.vector.tensor_copy(LHS[:], PT[:])
                    DI = workp.tile([128, 2048], BF16, tag="di")
                    ixb = IX0B[:, c, kg].rearrange("p s k -> p (s k)") \
                        .unsqueeze(2).broadcast_to([128, 16, 128])
                    io2 = IOTA2[:].rearrange("p (g f) -> p g f", g=16)
                    div = DI[:].rearrange("p (g f) -> p g f", g=16)
                    nc.gpsimd.tensor_tensor(div, io2, ixb, op=AO.subtract)
                    CL = workp.tile([128, 2048], BF16, tag="cl")
                    nc.vector.tensor_scalar(CL[:], DI[:], -1.5, 1.5,
                                            op0=AO.max, op1=AO.min)
                    PB = workp.tile([128, 2048], BF16, tag="pb")
                    nc.vector.tensor_tensor(PB[:], CL[:], DI[:],
                                            op=AO.not_equal)
                    return LHS, PB

                def stage_A(LHS, PB):
                    A = []
                    for s in range(2):
                        At = psA.tile([128, 1024], F32, tag=f"a{s}",
                                      name=f"A{s}")
                        A.append(At)
                    for s in range(2):
                        for h in range(2):
                            cs = slice(h * 512, (h + 1) * 512)
                            nc.tensor.matmul(
                                A[s][:, cs], LHS[s * 64:(s + 1) * 64, :],
                                RHS[s * 64:(s + 1) * 64, cs],
                                start=True, stop=False)
                            nc.tensor.matmul(
                                A[s][:, cs], BIGI[:],
                                PB[:, s * 1024 + h * 512:
                                    s * 1024 + (h + 1) * 512],
                                start=False, stop=True)
                    return A

                def stage_exp(A):
                    G = workp.tile([128, 2048], BF16, tag="g")
                    for s in range(2):
                        nc.scalar.activation(
                            G[:, s * 1024:(s + 1) * 1024], A[s][:],
                            AF.Exp, scale=-C)
                    return G

                PS_S = psS.tile([128, KG, 128], F32, tag="pss")

                def stage_s(G_, kg, c_):
                    Wv = G_[:].rearrange("p (s k f) -> p s k f", s=2, f=128)
                    first, last = c_ == 0, c_ == N_CHUNKS - 1
                    for k in range(KG):
                        nc.tensor.matmul(
                            PS_S[:, k, :], Wv[:, 0, k, :], Wv[:, 1, k, :],
                            start=first and (k % 4 == 0),
                            stop=last and (k % 4 == 3))

                def drain(kg):
                    OUT = outp.tile([128, KG, 128], F32, tag="out")
                    nc.vector.tensor_copy(OUT[:], PS_S[:])
                    nc.sync.dma_start(slab_d[a][:, kg * KG:(kg + 1) * KG, :],
                                      OUT[:])

                for i, (kg, c) in enumerate(passes):
                    LHS, PB = stage_mask(kg, c)
                    A = stage_A(LHS, PB)
                    G = stage_exp(A)
                    stage_s(G, kg, c)
                    if c == N_CHUNKS - 1:
                        drain(kg)
            if rep_ctx is not None:
                rep_ctx.__exit__(None, None, None)

    nc.finalize()
    return nc


def _host_prepare(inputs):
    rhs, iota2, ident = _host_consts()
    t_all = _host_tvals()
    lors = {"x": inputs["xlors"], "y": inputs["ylors"], "z": inputs["zlors"]}
    proj = {"x": inputs["xproj"], "y": inputs["yproj"], "z": inputs["zproj"]}
    base = {}
    for ai, a in enumerate(AXES):
        cols = ROTATIONS[a] + [i + 3 for i in ROTATIONS[a]]
        l = np.asarray(lors[a]).astype(np.float32)[:, cols]
        base[f"lors{ai}"] = np.ascontiguousarray(
            np.stack([l[:, 0], l[:, 1], l[:, 3], l[:, 4]]))
        base[f"proj{ai}"] = np.ascontiguousarray(
            np.asarray(proj[a]), dtype=np.float32)
    import ml_dtypes
    base["rhs64"] = rhs.astype(ml_dtypes.bfloat16)
    base["iota2"] = iota2.astype(ml_dtypes.bfloat16)
    base["ident"] = ident.astype(ml_dtypes.bfloat16)
    base["bigi"] = (256.0 * ident).astype(ml_dtypes.bfloat16)
    in_maps = []
    for cid in range(N_CORES):
        m = dict(base)
        tk = t_all[cid * N_K:(cid + 1) * N_K]
        m["tvals"] = np.broadcast_to(tk, (128, N_K)).copy()
        in_maps.append(m)
    return in_maps


def _host_gather(results):
    outs = []
    for ai, a in enumerate(AXES):
        bp = np.concatenate(
            [np.transpose(r[f"slab{ai}"], (0, 2, 1)) for r in results], axis=2)
        outs.append(np.ascontiguousarray(
            np.transpose(bp, BACK_ROTATIONS_IMAGE[a]).astype(np.float32)))
    return tuple(outs)


def kernel(image, xlors, ylors, zlors, xproj, yproj, zproj):
    from concourse.bass_utils import run_bass_kernel_spmd

    if "nc" not in _CACHE:
        _CACHE["nc"] = _build_kernel()
    nc = _CACHE["nc"]
    inputs = dict(xlors=np.asarray(xlors), ylors=np.asarray(ylors),
                  zlors=np.asarray(zlors), xproj=np.asarray(xproj),
                  yproj=np.asarray(yproj), zproj=np.asarray(zproj))
    in_maps = _host_prepare(inputs)
    res = run_bass_kernel_spmd(nc, in_maps, core_ids=list(range(N_CORES)))
    return _host_gather(res.results)



# revision 25
# speedup vs baseline: 1.6188x; 1.6188x over previous
"""PET tube-of-response backprojection on 8 TRN2 NeuronCores, v5.

Slice-sharded (core c owns 16 z-slices of all three backprojections; no
collective). Per-slice scatter computed as W_x^T @ W_y on the PE:

  A_s[p, kf]  = (f - u_s[p,k])^2 + 256*(|f - round(u)| >= 2)
  G_s         = Exp(-C*A_s [+ ln proj folded into coef])
  PSUM_S[k]  += Wx^T @ Wy

Per-pass structure (one 128-LOR chunk x 8 slices x 2 sides):
  DVE : DI = IOTA - v (pair-packed 2x), PB mask (fused ts 4x)
  HWDGE: LHS = CFP^T via DMA xbar transpose
  PE  : quad matmuls + penalty matmuls + 8 stage_s matmuls (psA 3-deep)
  ACT : 2 exps

Prep (coefficient slab per (axis, slice-group)) is pumped one op per
pass on the otherwise-idle Pool engine during the PREVIOUS axis's main
loop; only the very first prep is exposed (runs on DVE at startup).
"""

import math
import sys

sys.path.insert(0, "/opt/trn_rl_repo")
sys.path.insert(0, "/opt/trn_rl_repo/concourse")

import numpy as np

V = 1.5625
INV_V = float(np.float32(0.64))
SIGMA2 = 9.0 * math.pi / 4.0
C = 0.5 * V * V / SIGMA2          # exponent scale: w = exp(-C*(f-u)^2)
MAGIC = 12582912.0                # 1.5 * 2^23: round-to-nearest-even trick

N_CORES = 8
N_K = 16          # slices per core
KG = 8            # slices per pass (PSUM capacity)
N_CHUNKS = 128    # 128-LOR chunks
N_LORS = N_CHUNKS * 128

ROTATIONS = {"x": [1, 2, 0], "y": [0, 2, 1], "z": [0, 1, 2]}
BACK_ROTATIONS_IMAGE = {"x": [1, 2, 0], "y": [1, 0, 2], "z": [0, 1, 2]}
AXES = ("x", "y", "z")

PB_SCHEME = "abs_max"   # how the |DI|>=2 mask is fused on DVE

_CACHE = {}


def _host_consts():
    """Static tensors shared by all cores."""
    f = np.arange(128, dtype=np.float32)
    f2 = f * f
    f2hi = np.floor(f2 / 64.0) * 64.0          # exact bf16 (8-bit mantissa)
    f2lo = f2 - f2hi                           # 0..63, exact
    # RHS [64, 1024]: row r = k*8 + j, col = k'*128 + f_pos; nonzero iff k'==k
    pat = np.stack([f2hi, f2lo, f, f, f,
                    np.ones(128, np.float32),
                    np.ones(128, np.float32),
                    np.ones(128, np.float32)])  # [8, 128]
    rhs = np.zeros((128, 1024), np.float32)
    for k in range(8):
        rhs[k * 8:(k + 1) * 8, k * 128:(k + 1) * 128] = pat
    rhs[64:] = rhs[:64]
    # IOTA2: [128, 16, 128] -> flattened [128, 2048]: f pattern per block
    iota2 = np.broadcast_to(f, (128, 16, 128)).reshape(128, 2048).copy()
    bigi = 256.0 * np.broadcast_to(np.eye(128, dtype=np.float32),
                                   (128, 128)).copy()
    return rhs, iota2, bigi


def _host_tvals():
    zc = np.float32(-100.0) + (np.arange(128, dtype=np.float32)
                               + np.float32(0.5)) * np.float32(1.5625)
    return (zc + np.float32(100.0)) / np.float32(200.0)


def _build_kernel(repeat=1):
    from concourse import mybir, tile, bacc

    DT = mybir.dt
    F32 = DT.float32
    BF16 = DT.bfloat16
    AO = mybir.AluOpType
    AF = mybir.ActivationFunctionType
    n_axes = 3

    nc = bacc.Bacc("TRN2", target_bir_lowering=False, debug=False)
    lors_d = [nc.dram_tensor(f"lors{a}", [4, N_LORS], F32, kind="ExternalInput")
              for a in range(n_axes)]
    proj_d = [nc.dram_tensor(f"proj{a}", [N_LORS], F32, kind="ExternalInput")
              for a in range(n_axes)]
    rhs_d = nc.dram_tensor("rhs64", [128, 1024], BF16, kind="ExternalInput")
    iota2_d = nc.dram_tensor("iota2", [128, 2048], BF16, kind="ExternalInput")
    bigi_d = nc.dram_tensor("bigi", [128, 128], BF16, kind="ExternalInput")
    tval_d = nc.dram_tensor("tvals", [128, N_K], F32, kind="ExternalInput")
    slab_d = [nc.dram_tensor(f"slab{a}", [128, N_K, 128], F32,
                             kind="ExternalOutput") for a in range(n_axes)]

    with tile.TileContext(nc) as tc:
        with (
            tc.tile_pool(name="const", bufs=1) as constp,
            tc.tile_pool(name="coef", bufs=3) as coefp,
            tc.tile_pool(name="head", bufs=3) as headp,
            tc.tile_pool(name="pre", bufs=1) as prep,
            tc.tile_pool(name="work", bufs=2) as workp,
            tc.tile_pool(name="lhs", bufs=3) as lhsp,
            tc.tile_pool(name="out", bufs=1) as outp,
            tc.tile_pool(name="psA", bufs=3, space="PSUM") as psA,
            tc.tile_pool(name="psS", bufs=1, space="PSUM") as psS,
        ):
            # --- static tiles ---
            RHS = constp.tile([128, 1024], BF16, tag="rhs")
            nc.sync.dma_start(RHS[:], rhs_d[:])
            IOTA2 = constp.tile([128, 2048], BF16, tag="iota2")
            nc.sync.dma_start(IOTA2[:], iota2_d[:])
            BIGI = constp.tile([128, 128], BF16, tag="bigi")
            nc.sync.dma_start(BIGI[:], bigi_d[:])
            TT = constp.tile([128, N_K], F32, tag="tt")
            nc.sync.dma_start(TT[:], tval_d[:])

            heads = {}   # a -> (P1X, P1Y, P2X, P2Y, LNPC)
            slabs = {}   # (a, kg) -> (CFP, IXB)

            def emit_head(a, eng):
                """Per-axis input DMAs + -ln(proj)/C."""
                comp = []
                for r in range(4):
                    t_ = headp.tile([128, N_CHUNKS], F32, tag=f"comp{r}")
                    nc.sync.dma_start(
                        t_[:], lors_d[a][r, :].rearrange("(p c) -> p c", p=128))
                    comp.append(t_)
                yield
                PRJ = headp.tile([128, N_CHUNKS], F32, tag="prj")
                nc.sync.dma_start(PRJ[:],
                                  proj_d[a][:].rearrange("(p c) -> p c", p=128))
                yield
                LNPC = headp.tile([128, N_CHUNKS], F32, tag="lnpc")
                nc.scalar.activation(LNPC[:], PRJ[:], AF.Ln)
                yield
                eng.tensor_scalar(LNPC[:], LNPC[:], -1.0 / C, None,
                                  op0=AO.mult)
                heads[a] = (*comp, LNPC)
                yield

            def emit_prep(a, kg, eng, nparts=1):
                """Coefficient slab + ix0 table for (axis a, slice-group kg).
                Elementwise over chunks, so emitted in `nparts` c-slices to
                bound per-op latency when pumped on Pool."""
                P1X, P1Y, P2X, P2Y, LNPC = heads[a]
                CFP = coefp.tile([128, N_CHUNKS, 128], BF16, tag="cfp",
                                 name=f"cfp{a}k{kg}")
                IXB = coefp.tile([128, N_CHUNKS, 2, 8, 2], BF16, tag="ixb",
                                 name=f"ixb{a}k{kg}")
                cfpv = CFP[:].rearrange("p c (t k j) -> p c t k j", t=2, k=8)
                NP = N_CHUNKS // nparts
                tbf = TT[:, kg * 8:(kg + 1) * 8].unsqueeze(1) \
                    .broadcast_to([128, NP, 8])

                for s, (P1, P2) in enumerate(((P1X, P2X), (P1Y, P2Y))):
                    for part in range(nparts):
                        cs = slice(part * NP, (part + 1) * NP)
                        DX = prep.tile([128, NP], F32, tag="dx")
                        eng.tensor_tensor(DX[:], P2[:, cs], P1[:, cs],
                                          op=AO.subtract)
                        yield
                        dxb = DX[:].unsqueeze(2).broadcast_to([128, NP, 8])
                        p1b = P1[:, cs].unsqueeze(2) \
                            .broadcast_to([128, NP, 8])
                        T1 = prep.tile([128, NP, 8], F32, tag="t1")
                        T2 = prep.tile([128, NP, 8], F32, tag="t2")
                        T3 = prep.tile([128, NP, 8], F32, tag="t3")
                        T4 = prep.tile([128, NP, 8], F32, tag="t4")
                        CX = T1
                        eng.tensor_tensor(CX[:], tbf, dxb, op=AO.mult)
                        yield
                        eng.tensor_tensor(CX[:], CX[:], p1b, op=AO.add)
                        yield
                        # y = cx + 100 ; exact fp32 division by 1.5625 via
                        # q = y*0.64; r = ((y-q)-0.5q)-0.0625q; u' = q+r*0.64
                        Y_ = T2
                        eng.tensor_scalar(Y_[:], CX[:], 100.0, None,
                                          op0=AO.add)
                        yield
                        Q_ = T3
                        eng.tensor_scalar(Q_[:], Y_[:], INV_V, None,
                                          op0=AO.mult)
                        yield
                        # (Pool rejects scalar_tensor_tensor, so the fused
                        # a*s + b steps are split into ts-mult + tt-add)
                        T5 = prep.tile([128, NP, 8], F32, tag="t5")
                        R_ = T1
                        eng.tensor_tensor(R_[:], Y_[:], Q_[:], op=AO.subtract)
                        yield
                        eng.tensor_scalar(T5[:], Q_[:], -0.5, None,
                                          op0=AO.mult)
                        yield
                        eng.tensor_tensor(R_[:], T5[:], R_[:], op=AO.add)
                        yield
                        eng.tensor_scalar(T5[:], Q_[:], -0.0625, None,
                                          op0=AO.mult)
                        yield
                        eng.tensor_tensor(R_[:], T5[:], R_[:], op=AO.add)
                        yield
                        U = T2
                        eng.tensor_scalar(T5[:], R_[:], INV_V, None,
                                          op0=AO.mult)
                        yield
                        eng.tensor_tensor(U[:], T5[:], Q_[:], op=AO.add)
                        yield
                        eng.tensor_scalar(U[:], U[:], 0.5, None,
                                          op0=AO.subtract)
                        yield
                        # ix0 = round-to-even(u)
                        IX0 = T3
                        eng.tensor_scalar(IX0[:], U[:], MAGIC, MAGIC,
                                          op0=AO.add, op1=AO.subtract)
                        yield
                        for dup in range(2):
                            nc.scalar.copy(IXB[:, cs, s, :, dup], IX0[:])
                            yield
                        # q2 = u^2 (+ -ln(proj)/C on the y side)
                        Q2 = T3
                        eng.tensor_tensor(Q2[:], U[:], U[:], op=AO.mult)
                        yield
                        if s == 1:
                            lb = LNPC[:, cs].unsqueeze(2) \
                                .broadcast_to([128, NP, 8])
                            eng.tensor_tensor(Q2[:], Q2[:], lb, op=AO.add)
                            yield

                        # 3-way exact-residual bf16 splits of (-2u) and q2,
                        # written straight into the coefficient slab.
                        def split3(SRC, scratch, scale0, jbase):
                            eng.tensor_scalar(scratch[0][:], SRC[:], scale0,
                                              None, op0=AO.mult)
                            yield
                            cur, spare = scratch
                            for lvl in range(3):
                                # copies ride the ACT engine (Copy act.)
                                nc.scalar.copy(
                                    cfpv[:, cs, s, :, jbase + lvl], cur[:])
                                yield
                                if lvl == 2:
                                    break
                                eng.tensor_tensor(
                                    spare[:], cur[:],
                                    cfpv[:, cs, s, :, jbase + lvl],
                                    op=AO.subtract)
                                yield
                                cur, spare = spare, cur

                        yield from split3(U, (T4, T1), -2.0, 2)   # j = 2,3,4
                        yield from split3(Q2, (T4, T1), 1.0, 5)   # j = 5,6,7
                        # j = 0, 1: constant 1.0 rows (pair f2hi, f2lo)
                        eng.memset(cfpv[:, cs, s, :, 0:2], 1.0)
                        yield
                slabs[(a, kg)] = (CFP, IXB)

            # ---------- per-pass stages ----------
            def stage_mask(CFP, IXB, c):
                LHS = lhsp.tile([128, 128], BF16, tag="lhs")
                nc.sync.dma_start(LHS[:], CFP[:, c, :], transpose=True)
                DI = workp.tile([128, 2048], BF16, tag="di")
                ixb = IXB[:, c].rearrange("p s k two -> p (s k) two") \
                    .unsqueeze(2).broadcast_to([128, 16, 64, 2])
                io2 = IOTA2[:].rearrange("p (g f two) -> p g f two",
                                         g=16, two=2)
                div = DI[:].rearrange("p (g f two) -> p g f two",
                                      g=16, two=2)
                # subtract split DVE (pair-packed 2x) / Pool (1x, but idle);
                # Pool only supports plain arith tt, so clamp+compare go DVE
                nc.vector.tensor_tensor(div[:, 0:10], io2[:, 0:10],
                                        ixb[:, 0:10], op=AO.subtract)
                nc.gpsimd.tensor_tensor(div[:, 10:16], io2[:, 10:16],
                                        ixb[:, 10:16], op=AO.subtract)
                # PB = 1 iff |DI| >= 2 (exact small ints in bf16)
                CL = workp.tile([128, 2048], BF16, tag="cl")
                nc.vector.tensor_scalar(CL[:], DI[:], -1.5, 1.5,
                                        op0=AO.max, op1=AO.min)
                PB = workp.tile([128, 2048], BF16, tag="pb")
                nc.vector.tensor_tensor(PB[:], CL[:], DI[:],
                                        op=AO.not_equal)
                return LHS, PB

            def stage_exp(LHS, PB):
                G = workp.tile([128, 2048], BF16, tag="g")
                for s in range(2):
                    A = psA.tile([128, 1024], F32, tag="a", name=f"A{s}")
                    for h in range(2):
                        cs = slice(h * 512, (h + 1) * 512)
                        nc.tensor.matmul(
                            A[:, cs], LHS[s * 64:(s + 1) * 64, :],
                            RHS[s * 64:(s + 1) * 64, cs],
                            start=True, stop=False)
                        nc.tensor.matmul(
                            A[:, cs], BIGI[:],
                            PB[:, s * 1024 + h * 512:
                                s * 1024 + (h + 1) * 512],
                            start=False, stop=True)
                    nc.scalar.activation(
                        G[:, s * 1024:(s + 1) * 1024], A[:],
                        AF.Exp, scale=-C)
                return G

            # ---------- startup: axis 0 prep on DVE ----------
            for _ in emit_head(0, nc.vector):
                pass
            for kg in range(2):
                for _ in emit_prep(0, kg, nc.vector):
                    pass

            rep_ctx = tc.For_i(0, repeat, 1) if repeat > 1 else None
            if rep_ctx is not None:
                rep_ctx.__enter__()
            for a in range(n_axes):
                nxt = (a + 1) % n_axes

                def pump_gen_kg(b, kg_):
                    # the CFP generation for (b, kg1) only frees up once the
                    # (a, kg0) passes finish, so pump each kg of the next
                    # axis strictly during the SAME kg of the current axis —
                    # the pump then never blocks the Pool queue mid-pass.
                    if kg_ == 0:
                        yield from emit_head(b, nc.gpsimd)
                    yield from emit_prep(b, kg_, nc.gpsimd, nparts=1)

                for kg in range(2):
                    pump = pump_gen_kg(nxt, kg)
                    CFP, IXB = slabs[(a, kg)]
                    PS_S = psS.tile([128, KG, 128], F32, tag="pss")
                    for c in range(N_CHUNKS):
                        LHS, PB = stage_mask(CFP, IXB, c)
                        G = stage_exp(LHS, PB)
                        Wv = G[:].rearrange("p (s k f) -> p s k f",
                                            s=2, f=128)
                        first, last = c == 0, c == N_CHUNKS - 1
                        for k in range(KG):
                            nc.tensor.matmul(
                                PS_S[:, k, :], Wv[:, 0, k, :], Wv[:, 1, k, :],
                                start=first and (k % 4 == 0),
                                stop=last and (k % 4 == 3))
                        if c % 2 == 0:   # ~60 pump ops spread over 128 passes
                            next(pump, None)
                    OUT = outp.tile([128, KG, 128], F32, tag="out")
                    nc.scalar.copy(OUT[:], PS_S[:])
                    nc.sync.dma_start(slab_d[a][:, kg * KG:(kg + 1) * KG, :],
                                      OUT[:])
                    for _ in pump:   # leftovers (pump is sized to ~fit)
                        pass
            if rep_ctx is not None:
                rep_ctx.__exit__(None, None, None)

    nc.finalize()
    return nc


def _host_prepare(inputs):
    rhs, iota2, bigi = _host_consts()
    t_all = _host_tvals()
    lors = {"x": inputs["xlors"], "y": inputs["ylors"], "z": inputs["zlors"]}
    proj = {"x": inputs["xproj"], "y": inputs["yproj"], "z": inputs["zproj"]}
    base = {}
    for ai, a in enumerate(AXES):
        cols = ROTATIONS[a] + [i + 3 for i in ROTATIONS[a]]
        l = np.asarray(lors[a]).astype(np.float32)[:, cols]
        base[f"lors{ai}"] = np.ascontiguousarray(
            np.stack([l[:, 0], l[:, 1], l[:, 3], l[:, 4]]))
        base[f"proj{ai}"] = np.ascontiguousarray(
            np.asarray(proj[a]), dtype=np.float32)
    import ml_dtypes
    base["rhs64"] = rhs.astype(ml_dtypes.bfloat16)
    base["iota2"] = iota2.astype(ml_dtypes.bfloat16)
    base["bigi"] = bigi.astype(ml_dtypes.bfloat16)
    in_maps = []
    for cid in range(N_CORES):
        m = dict(base)
        tk = t_all[cid * N_K:(cid + 1) * N_K]
        m["tvals"] = np.broadcast_to(tk, (128, N_K)).copy()
        in_maps.append(m)
    return in_maps


def _host_gather(results):
    outs = []
    for ai, a in enumerate(AXES):
        bp = np.concatenate(
            [np.transpose(r[f"slab{ai}"], (0, 2, 1)) for r in results], axis=2)
        outs.append(np.ascontiguousarray(
            np.transpose(bp, BACK_ROTATIONS_IMAGE[a]).astype(np.float32)))
    return tuple(outs)


def kernel(image, xlors, ylors, zlors, xproj, yproj, zproj):
    from concourse.bass_utils import run_bass_kernel_spmd

    if "nc" not in _CACHE:
        _CACHE["nc"] = _build_kernel()
    nc = _CACHE["nc"]
    inputs = dict(xlors=np.asarray(xlors), ylors=np.asarray(ylors),
                  zlors=np.asarray(zlors), xproj=np.asarray(xproj),
                  yproj=np.asarray(yproj), zproj=np.asarray(zproj))
    in_maps = _host_prepare(inputs)
    res = run_bass_kernel_spmd(nc, in_maps, core_ids=list(range(N_CORES)))
    return _host_gather(res.results)
